# revision 23
# baseline (speedup 1.0000x reference)
"""ExtraMSAEmbedding Trainium2 kernel (V2).

out[s, r, :] = one_hot(msa[s, r], 23) @ W[:, :23].T
             + has_del[s, r] * W[:, 23] + del_val[s, r] * W[:, 24] + b

Strategy (8 NeuronCores, data-parallel over the 2048 extra sequences — 256
seqs = 98304 tokens per core):

- host sorts each core's tokens by msa class (stable argsort; inverse
  permutation applied while unsharding).  A 512-token PSUM bank then
  contains at most ONE class boundary (class sizes ~4300 >> 512), so the
  23-class lookup collapses to K=4 fp16 features per token:
  [has, del, step, ones] paired with per-bank weights
  [w23; w24; dW; base] where step = indicator of "past the boundary",
  dW = column delta across the boundary, base = b + W[:,c0].  The
  weights ride in the same DRAM rows as the features (slot-per-bank), so
  the per-core data layout is uniform and all 8 cores run ONE program.
- PE: two M=64 column-sets run concurrently (tokens 0..49151 -> PSUM
  partitions 0-63, rest -> 64-127).  Each set alternates between two
  32-row bands (tile_position) so each matmul's LDWEIGHTS hides under
  the previous matmul of the same set.  192 matmuls of [4,64]x[4,512].
- int8 output quantization (exact bound from W,b computed on host, scale
  folded into the weights): PSUM f32 -> int8 drains alternate between
  ScalarE (banks 0-3 per cycle, 2-bank chunks) and VectorE (banks 4-7),
  the only PSUM-capable engines.  Host dequantizes while unsharding.
- the whole per-core output (6.29MB) stages in a persistent SBUF buffer;
  output leaves as ~512KB SWDGE DMAs per PSUM cycle (tail cycles via the
  by-then-idle HWDGE queues).  Input streams over the two HWDGE queues in
  7 column chunks of small (<10KB) packets: the feature bands live on
  only 2 of the 16 SBUF AXI ports (~54GB/s), and an SDMA engine only
  bumps a DMA's completion sem after finishing its current packet, so
  small packets keep sem latency low and the PE starts ~4us after the
  framework preamble.  Measured ~46.5us vs 59.4us for the K=5/512-block
  predecessor; the body (12 PSUM cycles x ~2.6us) sits at the ACT+DVE
  drain floor, which is the binding constraint on TRN2 (fp32-only PSUM,
  1 elem/cycle/lane drains on the only two PSUM-capable engines).
"""

import numpy as np

N_SEQ, N_RES = 2048, 384
C_OUT = 64
N_CORES = 8
SEQ_PER_CORE = N_SEQ // N_CORES     # 256
T = SEQ_PER_CORE * N_RES            # 98304 tokens per core
NSET = 2                            # PE column sets (M=64 each)
SETTOK = T // NSET                  # 49152 tokens per set
BANK = 512                          # tokens per f32 PSUM bank
NBANK = SETTOK // BANK              # 96 logical banks (x2 sets each)
NCYC = NBANK // 8                   # 12 PSUM cycles
KDIM = 4                            # has, del, step, ones
NSLOT = NBANK // 2                  # 48 slots per band
SLOTW = C_OUT + BANK                # 576 cols/slot: [64 weights | 512 feats]
FCOLS = NSLOT * SLOTW               # 27648 row length
N_BLOCKS = NBANK                    # compat with test.py

_CACHE: dict = {}
_LAST_RESULT = None


def build_program(n_blocks: int = N_BLOCKS):
    """Build + compile the Bass/Tile program (same program for all cores)."""
    import concourse.bass as bass  # noqa: F401
    import concourse.mybir as mybir
    import concourse.tile as tile
    from concourse import bacc

    f32 = mybir.dt.float32
    f16 = mybir.dt.float16
    i8 = mybir.dt.int8

    nc = bacc.Bacc("TRN2", target_bir_lowering=False, debug=False)

    feat_d = nc.dram_tensor(
        "feat", [4, KDIM, FCOLS], f16, kind="ExternalInput"
    ).ap()
    out_d = nc.dram_tensor("out", [128, SETTOK], i8, kind="ExternalOutput").ap()

    with tile.TileContext(nc) as tc:
        with (
            tc.tile_pool(name="fpool", bufs=1) as fpool,
            tc.tile_pool(name="opool", bufs=1) as opool,
            tc.tile_pool(name="ppool", bufs=1, space=bass.MemorySpace.PSUM) as ppool,
        ):
            fsb = fpool.tile([128, FCOLS], f16, name="fsb")
            osb = opool.tile([128, SETTOK], i8, name="osb")
            pps = ppool.tile([128, 8 * BANK], f32, name="pps")
            _build_body(nc, fsb, osb, pps, feat_d, out_d)

    nc.compile()
    return nc


def _build_body(nc, fsb, osb, pps, feat_d, out_d):

    # input: the feature bands live on SBUF partitions {0-3,32-35} (AXI
    # port 0) and {64-67,96-99} (port 1), so input is port-limited to
    # ~54GB/s.  Stream it in column chunks (plane k of all 4 bands per
    # dma_start via the strided-partition AP).  Packets must stay small
    # (<=10KB): an SDMA engine only bumps a DMA's completion sem after it
    # round-robins past its current packet, so one long packet poisons
    # every later DMA's sem latency.  Everything rides the two HWDGE
    # queues (ready ~6us in): sync takes the head + bulk, scalar takes
    # two mid chunks and is free again before its first drain.
    # the ~600ns-per-issue queues are the head bottleneck (28 issues), so
    # spread chunks across all three DMA-capable queues: chunk 2 rides the
    # (until the first output, idle) gpsimd SWDGE queue, chunks 1/4 the
    # scalar HWDGE queue (idle until its first drain), the rest sync.
    CH = [0, 1152, 2304, 4608, 9216, 16128, 23040, FCOLS]
    QQ = {1: "scalar", 2: "gpsimd", 4: "scalar"}
    for ci in (2, 0, 1, 3, 4, 5, 6):
        eng = getattr(nc, QQ.get(ci, "sync"))
        for k in range(KDIM):
            eng.dma_start(
                fsb[k : 128 : 32, CH[ci] : CH[ci + 1]],
                feat_d[:, k, CH[ci] : CH[ci + 1]],
            )

    for c in range(NCYC):
        for o in range(8):
            gb = 8 * c + o
            bnd, i = gb % 2, gb // 2
            wc = i * SLOTW
            fc = wc + C_OUT
            for s in range(NSET):
                r0 = 32 * (2 * s + bnd)
                nc.tensor.matmul(
                    pps[64 * s : 64 * s + 64, o * BANK : (o + 1) * BANK],
                    fsb[r0 : r0 + KDIM, wc : wc + C_OUT],
                    fsb[r0 : r0 + KDIM, fc : fc + BANK],
                    tile_position=(r0, 64 * s),
                )
        # PSUM -> SBUF int8 drains (values pre-scaled via the weights;
        # f32 -> int8 converts round-to-nearest).  A/D interleaved across
        # the cycle so next-cycle PE refills hide under the other engine's
        # drain (period ~= DVE busy, fills off the critical path).
        base = 8 * c * BANK
        nc.scalar.copy(osb[:, base : base + 1024], pps[:, 0:1024])
        nc.vector.tensor_copy(osb[:, base + 1024 : base + 2048], pps[:, 1024:2048])
        nc.scalar.copy(osb[:, base + 2048 : base + 3072], pps[:, 2048:3072])
        nc.vector.tensor_copy(osb[:, base + 3072 : base + 4096], pps[:, 3072:4096])
        # output: SWDGE for the steady state; the tail cycles ride the (by
        # then idle) HWDGE queues to skip the gpsimd Q7 drain, the last one
        # split across both queues to halve the final transfer
        if c == NCYC - 1:
            nc.sync.dma_start(out_d[:, base : base + 2048], osb[:, base : base + 2048])
            nc.scalar.dma_start(
                out_d[:, base + 2048 : base + 4096], osb[:, base + 2048 : base + 4096]
            )
        elif c == NCYC - 2:
            nc.sync.dma_start(out_d[:, base : base + 4096], osb[:, base : base + 4096])
        else:
            nc.gpsimd.dma_start(out_d[:, base : base + 4096], osb[:, base : base + 4096])


def _prep_core(msa_c, has_c, del_c, W, b_, s_q):
    """Sort one core's tokens by class; build feature rows + weight slots."""
    f16 = np.float16
    perm = np.argsort(msa_c, kind="stable")
    cls = msa_c[perm]
    cb = cls.reshape(2 * NBANK, BANK)
    c0, c1 = cb[:, 0], cb[:, -1]
    # sorted stream + class sizes >> BANK  =>  at most one boundary per bank
    assert np.all((cb == c0[:, None]) | (cb == c1[:, None])), "bank w/ >2 classes"

    WT = W.T  # [25, 64]
    w4 = np.empty((2 * NBANK, KDIM, C_OUT), np.float32)
    w4[:, 0] = W[:, 23]
    w4[:, 1] = W[:, 24]
    w4[:, 2] = WT[c1] - WT[c0]
    w4[:, 3] = b_ + WT[c0]
    w4_16 = (w4 * s_q).astype(f16)

    has_b = has_c[perm].astype(f16).reshape(2 * NBANK, BANK)
    del_b = del_c[perm].astype(f16).reshape(2 * NBANK, BANK)
    step_b = (cb != c0[:, None]).astype(f16)

    planes = (has_b, del_b, step_b, None)  # None -> ones
    feat = np.empty((4, KDIM, NSLOT, SLOTW), f16)
    for s in (0, 1):
        for bnd in (0, 1):
            B_ = 2 * s + bnd
            sel = np.arange(NBANK * s + bnd, NBANK * (s + 1), 2)
            for k in range(KDIM):
                feat[B_, k, :, :C_OUT] = w4_16[sel][:, k]
                feat[B_, k, :, C_OUT:] = 1.0 if planes[k] is None else planes[k][sel]
    return perm, feat.reshape(4, KDIM, FCOLS)


def kernel(extra_msa, extra_has_deletion, extra_deletion_value, W, b):
    from concourse.bass_utils import run_bass_kernel_spmd

    f32 = np.float32
    msa = np.asarray(extra_msa)
    has_ = np.asarray(extra_has_deletion, dtype=f32)
    del_ = np.asarray(extra_deletion_value, dtype=f32)
    W = np.asarray(W, dtype=f32)
    b = np.asarray(b, dtype=f32)

    if "nc" not in _CACHE:
        _CACHE["nc"] = build_program()
    nc = _CACHE["nc"]

    # exact output bound for the int8 quantization scale
    Wb = W.T[:23] + b  # [23, 64]
    hi = Wb.max(0) + np.maximum(W[:, 23], 0) + np.maximum(W[:, 24], 0)
    lo = Wb.min(0) + np.minimum(W[:, 23], 0) + np.minimum(W[:, 24], 0)
    B = float(np.maximum(np.abs(hi), np.abs(lo)).max())
    s_q = 126.5 / B

    perms, in_maps = [], []
    for c in range(N_CORES):
        s0, s1 = c * SEQ_PER_CORE, (c + 1) * SEQ_PER_CORE
        perm, feat = _prep_core(
            np.ascontiguousarray(msa[s0:s1]).ravel(),
            np.ascontiguousarray(has_[s0:s1]).ravel(),
            np.ascontiguousarray(del_[s0:s1]).ravel(),
            W,
            b,
            s_q,
        )
        perms.append(perm)
        in_maps.append({"feat": feat})

    res = run_bass_kernel_spmd(nc, in_maps, list(range(N_CORES)))
    global _LAST_RESULT
    _LAST_RESULT = res

    # unshard: raw [128, 49152] int8 -> unsorted [256, 384, 64] f32
    inv_s = np.float32(1.0 / s_q)
    parts = []
    for c, r in enumerate(res.results):
        raw = r["out"]  # [128 (2x64 ch), SETTOK]
        srt = np.empty((T, C_OUT), f32)
        srt[:SETTOK] = raw[:64].T.astype(f32)
        srt[SETTOK:] = raw[64:].T.astype(f32)
        srt *= inv_s
        out_c = np.empty_like(srt)
        out_c[perms[c]] = srt
        parts.append(out_c.reshape(SEQ_PER_CORE, N_RES, C_OUT))
    return np.ascontiguousarray(np.concatenate(parts, axis=0))


# revision 24
# speedup vs baseline: 1.1608x; 1.1608x over previous
"""ExtraMSAEmbedding Trainium2 kernel (V2).

out[s, r, :] = one_hot(msa[s, r], 23) @ W[:, :23].T
             + has_del[s, r] * W[:, 23] + del_val[s, r] * W[:, 24] + b

Strategy (8 NeuronCores, data-parallel over the 2048 extra sequences — 256
seqs = 98304 tokens per core):

- host sorts each core's tokens by msa class (stable argsort; inverse
  permutation applied while unsharding).  A 512-token PSUM bank then
  contains at most ONE class boundary (class sizes ~4300 >> 512), so the
  23-class lookup collapses to K=4 fp16 features per token:
  [has, del, step, ones] paired with per-bank weights
  [w23; w24; dW; base] where step = indicator of "past the boundary",
  dW = column delta across the boundary, base = b + W[:,c0].  The
  weights ride in the same DRAM rows as the features (slot-per-bank), so
  the per-core data layout is uniform and all 8 cores run ONE program.
- PE: two M=64 column-sets run concurrently (tokens 0..49151 -> PSUM
  partitions 0-63, rest -> 64-127).  Each set alternates between two
  32-row bands (tile_position) so each matmul's LDWEIGHTS hides under
  the previous matmul of the same set.  192 matmuls of [4,64]x[4,512].
- int8 output quantization (exact bound from W,b computed on host, scale
  folded into the weights): PSUM f32 -> int8 drains alternate between
  ScalarE (banks 0-3 per cycle, 2-bank chunks) and VectorE (banks 4-7),
  the only PSUM-capable engines.  Host dequantizes while unsharding.
- the whole per-core output (6.29MB) stages in a persistent SBUF buffer;
  output leaves as ~512KB SWDGE DMAs per PSUM cycle (tail cycles via the
  by-then-idle HWDGE queues).  Input streams over the two HWDGE queues in
  7 column chunks of small (<10KB) packets: the feature bands live on
  only 2 of the 16 SBUF AXI ports (~54GB/s), and an SDMA engine only
  bumps a DMA's completion sem after finishing its current packet, so
  small packets keep sem latency low and the PE starts ~4us after the
  framework preamble.  Measured ~46.5us vs 59.4us for the K=5/512-block
  predecessor; the body (12 PSUM cycles x ~2.6us) sits at the ACT+DVE
  drain floor, which is the binding constraint on TRN2 (fp32-only PSUM,
  1 elem/cycle/lane drains on the only two PSUM-capable engines).
"""

import numpy as np

N_SEQ, N_RES = 2048, 384
C_OUT = 64
N_CORES = 8
SEQ_PER_CORE = N_SEQ // N_CORES     # 256
T = SEQ_PER_CORE * N_RES            # 98304 tokens per core
NSET = 2                            # PE column sets (M=64 each)
SETTOK = T // NSET                  # 49152 tokens per set
BANK = 512                          # tokens per f32 PSUM bank
NBANK = SETTOK // BANK              # 96 logical banks (x2 sets each)
NCYC = NBANK // 8                   # 12 PSUM cycles
KDIM = 4                            # has, del, step, ones
NSLOT = NBANK // 2                  # 48 slots per band
SLOTW = C_OUT + BANK                # 576 cols/slot: [64 weights | 512 feats]
FCOLS = NSLOT * SLOTW               # 27648 row length
N_BLOCKS = NBANK                    # compat with test.py

_CACHE: dict = {}
_LAST_RESULT = None


def build_program(n_blocks: int = N_BLOCKS):
    """Build + compile the Bass/Tile program (same program for all cores)."""
    import concourse.bass as bass  # noqa: F401
    import concourse.mybir as mybir
    import concourse.tile as tile
    from concourse import bacc

    f32 = mybir.dt.float32
    f16 = mybir.dt.float16
    i8 = mybir.dt.int8

    nc = bacc.Bacc("TRN2", target_bir_lowering=False, debug=False)

    feat_d = nc.dram_tensor(
        "feat", [4, KDIM, FCOLS], f16, kind="ExternalInput"
    ).ap()
    out_d = nc.dram_tensor("out", [128, SETTOK], i8, kind="ExternalOutput").ap()

    with tile.TileContext(nc) as tc:
        with (
            tc.tile_pool(name="fpool", bufs=1) as fpool,
            tc.tile_pool(name="opool", bufs=1) as opool,
            tc.tile_pool(name="ppool", bufs=1, space=bass.MemorySpace.PSUM) as ppool,
        ):
            fsb = fpool.tile([128, FCOLS], f16, name="fsb")
            osb = opool.tile([128, SETTOK], i8, name="osb")
            pps = ppool.tile([128, 8 * BANK], f32, name="pps")
            _build_body(nc, fsb, osb, pps, feat_d, out_d)

    nc.compile()
    return nc


def _build_body(nc, fsb, osb, pps, feat_d, out_d):

    # input: the feature bands live on SBUF partitions {0-3,32-35} (AXI
    # port 0) and {64-67,96-99} (port 1), so input is port-limited to
    # ~54GB/s.  Stream it in column chunks (plane k of all 4 bands per
    # dma_start via the strided-partition AP).  Packets must stay small
    # (<=10KB): an SDMA engine only bumps a DMA's completion sem after it
    # round-robins past its current packet, so one long packet poisons
    # every later DMA's sem latency.  Everything rides the two HWDGE
    # queues (ready ~6us in): sync takes the head + bulk, scalar takes
    # two mid chunks and is free again before its first drain.
    # chunk 0 covers the whole first PSUM cycle (slots 0-3) so cycle 0
    # gates on one sem; chunk 1 rides the (until its first drain, idle)
    # scalar HWDGE queue so cycles 0 and 1's features issue in parallel --
    # the 4 issues per chunk serialize at ~600ns each on a queue.
    CH = [0, 2304, 4608, 9216, 16128, 23040, FCOLS]
    for ci in range(len(CH) - 1):
        eng = nc.scalar if ci == 1 else nc.sync
        for k in range(KDIM):
            eng.dma_start(
                fsb[k : 128 : 32, CH[ci] : CH[ci + 1]],
                feat_d[:, k, CH[ci] : CH[ci + 1]],
            )

    for c in range(NCYC):
        for o in range(8):
            gb = 8 * c + o
            bnd, i = gb % 2, gb // 2
            wc = i * SLOTW
            fc = wc + C_OUT
            for s in range(NSET):
                r0 = 32 * (2 * s + bnd)
                nc.tensor.matmul(
                    pps[64 * s : 64 * s + 64, o * BANK : (o + 1) * BANK],
                    fsb[r0 : r0 + KDIM, wc : wc + C_OUT],
                    fsb[r0 : r0 + KDIM, fc : fc + BANK],
                    tile_position=(r0, 64 * s),
                )
        # PSUM -> SBUF int8 drains (values pre-scaled via the weights;
        # f32 -> int8 converts round-to-nearest).  A/D interleaved across
        # the cycle so next-cycle PE refills hide under the other engine's
        # drain (period ~= DVE busy, fills off the critical path).
        base = 8 * c * BANK
        nc.scalar.copy(osb[:, base : base + 1024], pps[:, 0:1024])
        nc.vector.tensor_copy(osb[:, base + 1024 : base + 2048], pps[:, 1024:2048])
        nc.scalar.copy(osb[:, base + 2048 : base + 3072], pps[:, 2048:3072])
        nc.vector.tensor_copy(osb[:, base + 3072 : base + 4096], pps[:, 3072:4096])
        # output: SWDGE for the steady state; the tail cycles ride the (by
        # then idle) HWDGE queues to skip the gpsimd Q7 drain, the last one
        # split across both queues to halve the final transfer
        if c == NCYC - 1:
            nc.sync.dma_start(out_d[:, base : base + 2048], osb[:, base : base + 2048])
            nc.scalar.dma_start(
                out_d[:, base + 2048 : base + 4096], osb[:, base + 2048 : base + 4096]
            )
        elif c == NCYC - 2:
            nc.sync.dma_start(out_d[:, base : base + 4096], osb[:, base : base + 4096])
        else:
            nc.gpsimd.dma_start(out_d[:, base : base + 4096], osb[:, base : base + 4096])


def _prep_core(msa_c, has_c, del_c, W, b_, s_q):
    """Sort one core's tokens by class; build feature rows + weight slots."""
    f16 = np.float16
    perm = np.argsort(msa_c, kind="stable")
    cls = msa_c[perm]
    cb = cls.reshape(2 * NBANK, BANK)
    c0, c1 = cb[:, 0], cb[:, -1]
    # sorted stream + class sizes >> BANK  =>  at most one boundary per bank
    assert np.all((cb == c0[:, None]) | (cb == c1[:, None])), "bank w/ >2 classes"

    WT = W.T  # [25, 64]
    w4 = np.empty((2 * NBANK, KDIM, C_OUT), np.float32)
    w4[:, 0] = W[:, 23]
    w4[:, 1] = W[:, 24]
    w4[:, 2] = WT[c1] - WT[c0]
    w4[:, 3] = b_ + WT[c0]
    w4_16 = (w4 * s_q).astype(f16)

    has_b = has_c[perm].astype(f16).reshape(2 * NBANK, BANK)
    del_b = del_c[perm].astype(f16).reshape(2 * NBANK, BANK)
    step_b = (cb != c0[:, None]).astype(f16)

    planes = (has_b, del_b, step_b, None)  # None -> ones
    feat = np.empty((4, KDIM, NSLOT, SLOTW), f16)
    for s in (0, 1):
        for bnd in (0, 1):
            B_ = 2 * s + bnd
            sel = np.arange(NBANK * s + bnd, NBANK * (s + 1), 2)
            for k in range(KDIM):
                feat[B_, k, :, :C_OUT] = w4_16[sel][:, k]
                feat[B_, k, :, C_OUT:] = 1.0 if planes[k] is None else planes[k][sel]
    return perm, feat.reshape(4, KDIM, FCOLS)


def kernel(extra_msa, extra_has_deletion, extra_deletion_value, W, b):
    from concourse.bass_utils import run_bass_kernel_spmd

    f32 = np.float32
    msa = np.asarray(extra_msa)
    has_ = np.asarray(extra_has_deletion, dtype=f32)
    del_ = np.asarray(extra_deletion_value, dtype=f32)
    W = np.asarray(W, dtype=f32)
    b = np.asarray(b, dtype=f32)

    if "nc" not in _CACHE:
        _CACHE["nc"] = build_program()
    nc = _CACHE["nc"]

    # exact output bound for the int8 quantization scale
    Wb = W.T[:23] + b  # [23, 64]
    hi = Wb.max(0) + np.maximum(W[:, 23], 0) + np.maximum(W[:, 24], 0)
    lo = Wb.min(0) + np.minimum(W[:, 23], 0) + np.minimum(W[:, 24], 0)
    B = float(np.maximum(np.abs(hi), np.abs(lo)).max())
    s_q = 126.5 / B

    perms, in_maps = [], []
    for c in range(N_CORES):
        s0, s1 = c * SEQ_PER_CORE, (c + 1) * SEQ_PER_CORE
        perm, feat = _prep_core(
            np.ascontiguousarray(msa[s0:s1]).ravel(),
            np.ascontiguousarray(has_[s0:s1]).ravel(),
            np.ascontiguousarray(del_[s0:s1]).ravel(),
            W,
            b,
            s_q,
        )
        perms.append(perm)
        in_maps.append({"feat": feat})

    res = run_bass_kernel_spmd(nc, in_maps, list(range(N_CORES)))
    global _LAST_RESULT
    _LAST_RESULT = res

    # unshard: raw [128, 49152] int8 -> unsorted [256, 384, 64] f32
    inv_s = np.float32(1.0 / s_q)
    parts = []
    for c, r in enumerate(res.results):
        raw = r["out"]  # [128 (2x64 ch), SETTOK]
        srt = np.empty((T, C_OUT), f32)
        srt[:SETTOK] = raw[:64].T.astype(f32)
        srt[SETTOK:] = raw[64:].T.astype(f32)
        srt *= inv_s
        out_c = np.empty_like(srt)
        out_c[perms[c]] = srt
        parts.append(out_c.reshape(SEQ_PER_CORE, N_RES, C_OUT))
    return np.ascontiguousarray(np.concatenate(parts, axis=0))
